# revision 19
# baseline (speedup 1.0000x reference)
"""Trainium2 Bass kernel for nn_Din_attention2 (sparse_attention).

Pure data parallel over batch: B=262144 items sharded 8 ways (32768/core).

Per-item math (see reference):
    q  = relu(query @ Wq + bq)                       [8]
    z1 = q @ W1q + x_s @ W1x + b1    (din folding)   [80]  per s
    h1 = sigmoid(z1); z2 = h1 @ W2 + b2; h2 = sigmoid(z2)  [40]
    sc_s = h2 @ W3 (+const, dropped: softmax-invariant)
    w = masked softmax(sc); out = sum_s w_s * x_s

sigmoid folded to tanh (one ACT table set for tanh only):
    sigmoid(z) = 0.5*tanh(0.5*z) + 0.5, absorbed into next layer's
    weights/biases on the host. softmax exp computed via
    e^d = (1+tanh(d/2)) / (1-tanh(d/2)).

Layout: item = p*256 + g (partition-major), g = sb*16 + j.
Superblock = 2048 items = 16 chunks of 128. Feature-major MLP via PE
transposes; x padded to 32 f32/s so transposed s-slices land on
32-aligned partitions (matmul tile_position requirement). All 5 scores
land in one PSUM bank at rows {0,8,16,24,32} via zero-padded-M (M=33)
stationary operands, then one PE transpose per chunk returns them
batch-major for the masked softmax on DVE.
"""

import sys

for _p in ("/opt/trn_rl_repo", "/root/.axon_site/_ro/trn_rl_repo"):
    if _p not in sys.path:
        sys.path.append(_p)

import numpy as np

import concourse.bass as bass
import concourse.bacc as bacc
import concourse.tile as tile
from concourse import mybir
from concourse.bass_utils import run_bass_kernel_spmd
from concourse.masks import make_identity

F32 = mybir.dt.float32
I32 = mybir.dt.int32
ALU = mybir.AluOpType
AF = mybir.ActivationFunctionType

B, S, D, Q = 262144, 5, 8, 64
NCORES = 8
BC = B // NCORES            # 32768 items per core
GP = BC // 128              # 256 items per partition
SBI = 2048                  # superblock items
NSB = BC // SBI             # 16 superblocks
GSB = SBI // 128            # 16 g-slots per superblock
NEG = float(np.float32(-(2.0 ** 32) + 1.0))

_BUILT = None


def _fold_weights(Wq, bq, W1, b1, W2, b2, W3, b3):
    """Host-side weight folding. Returns dict of device weight arrays."""
    Wq = np.asarray(Wq, np.float32)
    bq = np.asarray(bq, np.float32)
    W1 = np.asarray(W1, np.float32)
    b1 = np.asarray(b1, np.float32)
    W2 = np.asarray(W2, np.float32)
    b2 = np.asarray(b2, np.float32)
    W3 = np.asarray(W3, np.float32)

    # din = [q, x, q-x, q-x] @ W1  ->  q @ W1q + x @ W1x
    W1a, W1b, W1c, W1d = W1[0:8], W1[8:16], W1[16:24], W1[24:32]
    W1q = W1a + W1c + W1d           # [8, 80]
    W1x = W1b - W1c - W1d           # [8, 80]

    # sigmoid -> tanh folding
    b1h = 0.5 * b1                                    # t1 = tanh(0.5*z1): bias
    W2h = 0.5 * W2                                    # [80, 40]
    b2h = 0.5 * (b2 + 0.5 * W2.sum(axis=0))           # [40]
    W3h = 0.5 * W3[:, 0]                              # [40]; +const dropped

    # replicated / padded layouts. Per 32-partition group g: rows 0:8 carry
    # x_{s=g}, rows 8:16 carry q (Wq replicas land there via MMq), rows
    # 16:24 of group 0 carry x_{s=4}.
    wqrep = np.zeros((64, 128), np.float32)
    for g in range(4):
        wqrep[:, 32 * g + 8: 32 * g + 16] = Wq
    bqmix = np.zeros((128, 1), np.float32)
    minclip = np.full((128, 1), -3.0e38, np.float32)
    for g in range(4):
        bqmix[32 * g + 8: 32 * g + 16, 0] = bq
        minclip[32 * g + 8: 32 * g + 16, 0] = 0.0
    w1mix = np.zeros((128, 80), np.float32)
    for g in range(4):
        w1mix[32 * g: 32 * g + 8] = W1x
        w1mix[32 * g + 8: 32 * g + 16] = W1q
    w1m4 = np.zeros((24, 80), np.float32)
    w1m4[8:16] = W1q
    w1m4[16:24] = W1x
    b2rep = np.zeros((128, 1), np.float32)
    b2rep[0:40, 0] = b2h
    b2rep[64:104, 0] = b2h
    # K-paired score weights: pair t covers s=2t (rows 0:40, score col
    # 16t) and s=2t+1 (rows 64:104, score col 16t+8); w3p2[:, 2] = s4.
    w3p2 = np.zeros((128, 3, 33), np.float32)
    for s in range(5):
        t, odd = divmod(s, 2)
        base = 64 if odd else 0
        w3p2[base:base + 40, t, 8 * s] = W3h

    return {
        "wqrep": wqrep,
        "bqmix": bqmix,
        "minclip": minclip,
        "w1mix": w1mix,
        "w1m4": w1m4,
        "w2h": np.ascontiguousarray(W2h),
        "b1h": np.ascontiguousarray(b1h.reshape(80, 1)),
        "b2rep": b2rep,
        "w3p2": w3p2,
    }


def _build():
    nc = bacc.Bacc(trn_type="TRN2")

    d_query = nc.dram_tensor("query", [BC, Q], F32, kind="ExternalInput")
    d_inputs = nc.dram_tensor("inputs", [BC, S * D], F32, kind="ExternalInput")
    d_mask = nc.dram_tensor("mask", [BC], I32, kind="ExternalInput")
    d_wqrep = nc.dram_tensor("wqrep", [64, 128], F32, kind="ExternalInput")
    d_bqmix = nc.dram_tensor("bqmix", [128, 1], F32, kind="ExternalInput")
    d_minclip = nc.dram_tensor("minclip", [128, 1], F32, kind="ExternalInput")
    d_w1mix = nc.dram_tensor("w1mix", [128, 80], F32, kind="ExternalInput")
    d_w1m4 = nc.dram_tensor("w1m4", [24, 80], F32, kind="ExternalInput")
    d_w2h = nc.dram_tensor("w2h", [80, 40], F32, kind="ExternalInput")
    d_b1h = nc.dram_tensor("b1h", [80, 1], F32, kind="ExternalInput")
    d_b2rep = nc.dram_tensor("b2rep", [128, 1], F32, kind="ExternalInput")
    d_w3p2 = nc.dram_tensor("w3p2", [128, 3, 33], F32, kind="ExternalInput")
    d_out = nc.dram_tensor("out", [BC, D], F32, kind="ExternalOutput")
    d_scores = nc.dram_tensor("scores", [BC, S], F32, kind="ExternalOutput")

    q_view = d_query[:, :].rearrange("(p g) q -> p g q", p=128)       # [128,256,64]
    x_view = d_inputs[:, :].rearrange("(p g) f -> p g f", p=128)      # [128,256,40]
    m_view = d_mask[:].rearrange("(p g) -> p g", p=128)               # [128,256]
    o_view = d_out[:, :].rearrange("(p g) d -> p g d", p=128)         # [128,256,8]
    s_view = d_scores[:, :].rearrange("(p g) s -> p g s", p=128)      # [128,256,5]

    with tile.TileContext(nc) as tc:
        with (
            tc.tile_pool(name="const", bufs=1) as cpool,
            tc.tile_pool(name="io", bufs=2) as iop,
            tc.tile_pool(name="feat", bufs=2) as fp,
            tc.tile_pool(name="big", bufs=1) as bigp,
            tc.tile_pool(name="t2", bufs=6) as t2pool,
            tc.tile_pool(name="soft", bufs=2) as softp,
            tc.tile_pool(name="pA", bufs=2, space="PSUM") as pA,
            tc.tile_pool(name="pB", bufs=3, space="PSUM") as pB,
        ):
            # ---- one-time constants ----
            ident = cpool.tile([128, 128], F32)
            make_identity(nc, ident)
            wqrep = cpool.tile([64, 128], F32)
            nc.sync.dma_start(out=wqrep, in_=d_wqrep[:, :])
            bqmix = cpool.tile([128, 1], F32)
            nc.sync.dma_start(out=bqmix, in_=d_bqmix[:, :])
            minclip = cpool.tile([128, 1], F32)
            nc.sync.dma_start(out=minclip, in_=d_minclip[:, :])
            w1mix = cpool.tile([128, 80], F32)
            nc.sync.dma_start(out=w1mix, in_=d_w1mix[:, :])
            w1m4 = cpool.tile([24, 80], F32)
            nc.sync.dma_start(out=w1m4, in_=d_w1m4[:, :])
            w2h = cpool.tile([80, 40], F32)
            nc.sync.dma_start(out=w2h, in_=d_w2h[:, :])
            b1h = cpool.tile([80, 1], F32)
            nc.sync.dma_start(out=b1h, in_=d_b1h[:, :])
            b2rep = cpool.tile([128, 1], F32)
            nc.sync.dma_start(out=b2rep, in_=d_b2rep[:, :])
            w3p2 = cpool.tile([128, 3, 33], F32)
            nc.sync.dma_start(out=w3p2, in_=d_w3p2[:, :, :])
            # two persistent, pre-zeroed x staging tiles (ping-pong): only
            # the real lanes get overwritten each superblock, pad stays 0
            xpads = [cpool.tile([128, GSB, 4, 32], F32, name=f"xpad{i}",
                                tag=f"xpad{i}")
                     for i in range(2)]
            nc.vector.memset(xpads[0], 0.0)
            nc.vector.memset(xpads[1], 0.0)
            maskc = cpool.tile([128, GP], I32)
            nc.sync.dma_start(out=maskc, in_=m_view)
            i5g = cpool.tile([128, GSB, 5], F32)
            nc.gpsimd.iota(
                i5g, pattern=[[0, GSB], [1, 5]], base=0,
                channel_multiplier=0, allow_small_or_imprecise_dtypes=True,
            )
            # re-produce on DVE so hot-loop DVE consumers never need a
            # cross-engine wait (the TT ISA struct has one wait slot)
            i5f = cpool.tile([128, GSB, 5], F32)
            nc.vector.tensor_copy(out=i5f, in_=i5g)
            out_all = cpool.tile([128, GP, D], F32)
            sc_all = cpool.tile([128, GP, S], F32)

            # one-time wait absorbers: several ISA structs (matmul LDW, DVE
            # tensor-tensor) have a single sync-wait slot, so hot-loop
            # instructions must never need BOTH a constant-producer wait and
            # a data wait. Touch every constant once per consuming engine so
            # the engines' vector clocks already cover them.
            psdum = pA.tile([128, 512], F32, tag="pA")
            nc.tensor.matmul(psdum[0:1, 0:1], lhsT=ident[:, 0:1],
                             rhs=ident[:, 0:1], start=True, stop=True)
            nc.tensor.matmul(psdum[0:1, 1:2], lhsT=wqrep[0:64, 0:1],
                             rhs=ident[0:64, 0:1], start=True, stop=True)
            nc.tensor.matmul(psdum[0:1, 2:3], lhsT=w1mix[0:16, 0:1],
                             rhs=ident[0:16, 0:1], start=True, stop=True)
            nc.tensor.matmul(psdum[0:1, 3:4], lhsT=w1m4[0:24, 0:1],
                             rhs=ident[0:24, 0:1], start=True, stop=True)
            nc.tensor.matmul(psdum[0:1, 4:5], lhsT=w2h[:, 0:1],
                             rhs=ident[0:80, 0:1], start=True, stop=True)
            nc.tensor.matmul(psdum[0:1, 5:6], lhsT=w3p2[0:104, 0, 0:1],
                             rhs=ident[0:104, 0:1], start=True, stop=True)
            dscr = cpool.tile([128, 8], F32, name="dscr", tag="dscr")
            nc.vector.tensor_copy(out=dscr[:, 0:1], in_=bqmix)
            nc.vector.tensor_copy(out=dscr[:, 1:2], in_=minclip)
            # read the dummy psum so its pool slot is released by a DVE
            # access (PE-WAR on the slot would cost a second wait slot)
            nc.vector.tensor_copy(out=dscr[0:1, 2:8], in_=psdum[0:1, 0:6])
            ascr = cpool.tile([128, 4], F32, name="ascr", tag="ascr")
            nc.scalar.copy(out=ascr[0:80, 0:1], in_=b1h)
            nc.scalar.copy(out=ascr[:, 1:2], in_=b2rep)

            for sb in range(NSB):
                g0 = sb * GSB
                # ---- loads ----
                qx = iop.tile([128, GSB, 64], F32, tag="qx")
                nc.sync.dma_start(out=qx, in_=q_view[:, g0:g0 + GSB, :])
                xn = iop.tile([128, GSB, S, D], F32, tag="xn")
                nc.sync.dma_start(
                    out=xn,
                    in_=x_view[:, g0:g0 + GSB, :].rearrange(
                        "p g (s d) -> p g s d", s=S),
                )
                # x_{0..3} padded to 32 f32 per s so transposed slices are
                # 32-partition aligned; x_4 goes to group-0 lanes 16:24
                xpad = xpads[sb % 2]
                nc.vector.tensor_copy(out=xpad[:, :, :, 0:8], in_=xn[:, :, 0:4, :])
                nc.vector.tensor_copy(out=xpad[:, :, 0, 16:24], in_=xn[:, :, 4, :])

                # ---- transposes to feature-major + query MLP head ----
                # xq rows per group g: [x_g(0:8); q(8:16)], group 0 also
                # x_4 at 16:24. MMq writes q rows (start=True over the whole
                # bank), the x transposes then accumulate on top.
                qxT = fp.tile([64, 4, 512], F32, tag="qxT")
                xq = fp.tile([128, 4, 512], F32, tag="xq")
                for n in range(4):
                    psq = pA.tile([128, 512], F32, tag="pA")
                    for j in range(4):
                        c = 4 * n + j
                        nc.tensor.transpose(
                            psq[0:64, 128 * j:128 * (j + 1)], qx[:, c, :], ident)
                    nc.vector.tensor_copy(out=qxT[:, n, :], in_=psq[0:64, :])
                    psmix = pA.tile([128, 512], F32, tag="pA")
                    nc.tensor.matmul(
                        psmix, lhsT=wqrep, rhs=qxT[:, n, :],
                        start=True, stop=False, skip_group_check=True)
                    for j in range(4):
                        c = 4 * n + j
                        nc.tensor.matmul(
                            psmix[:, 128 * j:128 * (j + 1)],
                            lhsT=xpad[:, c, :, :].rearrange("p a b -> p (a b)"),
                            rhs=ident, is_transpose=True,
                            start=False, stop=(j == 3), skip_group_check=True)
                    # relu+bias applied to the q rows only (per-partition
                    # bias / lower-clip vectors)
                    nc.vector.tensor_scalar(
                        out=xq[:, n, :], in0=psmix, scalar1=bqmix,
                        scalar2=minclip, op0=ALU.add, op1=ALU.max)

                # ---- layer 1: t1 = tanh(0.5*z1 + 0.5*b1) ----
                t1 = bigp.tile([80, S, 4, 512], F32, tag="t1")
                for s in range(S):
                    gs = (s % 4) * 32
                    for np_ in range(2):
                        ph1 = pB.tile([128, 2, 512], F32, tag="pB")
                        for nn in range(2):
                            n = 2 * np_ + nn
                            if s < 4:
                                nc.tensor.matmul(
                                    ph1[0:80, nn, :],
                                    lhsT=w1mix[gs:gs + 16, :],
                                    rhs=xq[gs:gs + 16, n, :],
                                    start=True, stop=True,
                                    tile_position=(gs, 0))
                            else:
                                nc.tensor.matmul(
                                    ph1[0:80, nn, :],
                                    lhsT=w1m4,
                                    rhs=xq[0:24, n, :],
                                    start=True, stop=True,
                                    tile_position=(0, 0))
                        nc.scalar.activation(
                            out=t1[:, s, 2 * np_:2 * np_ + 2, :],
                            in_=ph1[0:80, :, :],
                            func=AF.Tanh, bias=b1h, scale=0.5)

                # ---- layer 2 + scores, n-pair at a time ----
                scb = softp.tile([33, 4, 512], F32, tag="scb")
                for np_ in range(2):
                    t2tiles = []
                    for sp in ((0, 1), (2, 3), (4,)):
                        ph2 = pB.tile([128, 2, 512], F32, tag="pB")
                        for nn in range(2):
                            n = 2 * np_ + nn
                            nc.tensor.matmul(
                                ph2[0:40, nn, :], lhsT=w2h,
                                rhs=t1[:, sp[0], n, :], start=True, stop=True)
                            if len(sp) > 1:
                                nc.tensor.matmul(
                                    ph2[64:104, nn, :], lhsT=w2h,
                                    rhs=t1[:, sp[1], n, :], start=True, stop=True)
                        t2p = t2pool.tile([128, 2, 512], F32, tag="t2p")
                        nc.scalar.activation(
                            out=t2p, in_=ph2, func=AF.Tanh, bias=b2rep, scale=0.5)
                        t2tiles.append(t2p)

                    for nn in range(2):
                        n = 2 * np_ + nn
                        pssc = pA.tile([128, 512], F32, tag="pA")
                        nc.tensor.matmul(
                            pssc[0:33, :], lhsT=w3p2[0:104, 0, :],
                            rhs=t2tiles[0][0:104, nn, :],
                            start=True, stop=False)
                        nc.tensor.matmul(
                            pssc[0:33, :], lhsT=w3p2[0:104, 1, :],
                            rhs=t2tiles[1][0:104, nn, :],
                            start=False, stop=False)
                        nc.tensor.matmul(
                            pssc[0:33, :], lhsT=w3p2[0:40, 2, :],
                            rhs=t2tiles[2][0:40, nn, :],
                            start=False, stop=True)
                        nc.vector.tensor_copy(out=scb[:, n, :], in_=pssc[0:33, :])

                # ---- scores back to batch-major: psum rows {8s} -> cols ----
                scT = []
                for t in range(2):
                    ps = pA.tile([128, 512], F32, tag="pA")
                    for cc in range(8):
                        c = 8 * t + cc
                        nc.tensor.transpose(
                            ps[:, 40 * cc:40 * cc + 33],
                            scb[:, c // 4, 128 * (c % 4):128 * (c % 4 + 1)],
                            ident[0:33, 0:33])
                    scT.append(ps)
                sc_t = softp.tile([128, GSB, 5], F32, tag="sc_t")
                for t in range(2):
                    nc.vector.tensor_copy(
                        out=sc_t[:, 8 * t:8 * t + 8, :],
                        in_=scT[t][:, 0:320].rearrange(
                            "p (c s k) -> p c s k", c=8, s=5)[:, :, :, 0:1]
                        .rearrange("p c s k -> p c (s k)"),
                    )

                # ---- masked softmax (batch-major, DVE + one tanh) ----
                maskf = softp.tile([128, GSB], F32, tag="maskf")
                nc.vector.tensor_copy(out=maskf, in_=maskc[:, g0:g0 + GSB])
                km = softp.tile([128, GSB, 5], mybir.dt.uint8, tag="km")
                nc.vector.tensor_tensor(
                    out=km, in0=i5f,
                    in1=maskf[:, :, None].broadcast_to((128, GSB, 5)),
                    op=ALU.is_lt)
                msk = softp.tile([128, GSB, 5], F32, tag="msk")
                nc.vector.memset(msk, NEG)
                nc.vector.copy_predicated(out=msk, mask=km, data=sc_t)
                mx = softp.tile([128, GSB], F32, tag="mx")
                nc.vector.tensor_reduce(
                    out=mx, in_=msk, axis=mybir.AxisListType.X, op=ALU.max)
                dd = softp.tile([128, GSB, 5], F32, tag="dd")
                nc.vector.tensor_tensor(
                    out=dd, in0=msk,
                    in1=mx[:, :, None].broadcast_to((128, GSB, 5)),
                    op=ALU.subtract)
                # e^d = (1+tanh(d/2))/(1-tanh(d/2)); keeps ACT on one table set
                th = softp.tile([128, GSB, 5], F32, tag="th")
                nc.scalar.activation(out=th, in_=dd, func=AF.Tanh, scale=0.5)
                uu = softp.tile([128, GSB, 5], F32, tag="uu")
                nc.vector.tensor_scalar(
                    out=uu, in0=th, scalar1=1.0, scalar2=None, op0=ALU.add)
                vv = softp.tile([128, GSB, 5], F32, tag="vv")
                nc.vector.tensor_scalar(
                    out=vv, in0=th, scalar1=-1.0, scalar2=1.0,
                    op0=ALU.mult, op1=ALU.add)
                nc.vector.reciprocal(out=vv, in_=vv)
                ee = softp.tile([128, GSB, 5], F32, tag="ee")
                nc.vector.tensor_tensor(out=ee, in0=uu, in1=vv, op=ALU.mult)
                den = softp.tile([128, GSB], F32, tag="den")
                nc.vector.tensor_reduce(
                    out=den, in_=ee, axis=mybir.AxisListType.X, op=ALU.add)
                nc.vector.reciprocal(out=den, in_=den)
                nc.vector.tensor_tensor(
                    out=sc_all[:, g0:g0 + GSB, :], in0=ee,
                    in1=den[:, :, None].broadcast_to((128, GSB, 5)),
                    op=ALU.mult)

                # ---- output: out[p,g,d] = sum_s w[p,g,s] * x[p,g,s,d] ----
                wx = softp.tile([128, GSB, S, D], F32, tag="wx")
                nc.vector.tensor_tensor(
                    out=wx, in0=xn,
                    in1=sc_all[:, g0:g0 + GSB, :, None].broadcast_to(
                        (128, GSB, S, D)),
                    op=ALU.mult)
                nc.vector.tensor_reduce(
                    out=out_all[:, g0:g0 + GSB, :],
                    in_=wx.rearrange("p g s d -> p g d s"),
                    axis=mybir.AxisListType.X, op=ALU.add)

            nc.sync.dma_start(out=o_view, in_=out_all)
            nc.sync.dma_start(out=s_view, in_=sc_all)

    nc.finalize()
    return nc


def _get_built():
    global _BUILT
    if _BUILT is None:
        _BUILT = _build()
    return _BUILT


def _run(inputs_np, trace=False):
    nc = _get_built()
    w = _fold_weights(
        inputs_np["Wq"], inputs_np["bq"], inputs_np["W1"], inputs_np["b1"],
        inputs_np["W2"], inputs_np["b2"], inputs_np["W3"], inputs_np["b3"])
    query = np.ascontiguousarray(
        np.asarray(inputs_np["query"], np.float32).reshape(B, Q))
    xins = np.ascontiguousarray(
        np.asarray(inputs_np["inputs"], np.float32).reshape(B, S * D))
    mask = np.ascontiguousarray(np.asarray(inputs_np["mask"], np.int32))

    in_maps = []
    for c in range(NCORES):
        sl = slice(c * BC, (c + 1) * BC)
        m = {"query": query[sl], "inputs": xins[sl], "mask": mask[sl]}
        m.update(w)
        in_maps.append(m)

    kwargs = {}
    if trace:
        kwargs = dict(trace=True, trace_cores=[0])
    res = run_bass_kernel_spmd(nc, in_maps, core_ids=list(range(NCORES)), **kwargs)
    out = np.concatenate([r["out"] for r in res.results], axis=0)
    scores = np.concatenate([r["scores"] for r in res.results], axis=0)
    return out.reshape(B, 1, D), scores.reshape(B, 1, S), res


def kernel(**inputs):
    out, scores, _ = _run(inputs, trace=False)
    return out, scores


def bench(inputs_np, iters=30):
    """Time repeated on-device executions (inputs device_put once).

    Returns (best_ns, times_ns). Mirrors bass2jax.run_bass_via_pjrt's
    sharded path without donation so the executable can be re-run.
    """
    import time

    import jax
    from jax.experimental.shard_map import shard_map
    from jax.sharding import Mesh, PartitionSpec

    from concourse import bass2jax, mybir as _mybir

    nc = _get_built()
    bass2jax.install_neuronx_cc_hook()

    w = _fold_weights(
        inputs_np["Wq"], inputs_np["bq"], inputs_np["W1"], inputs_np["b1"],
        inputs_np["W2"], inputs_np["b2"], inputs_np["W3"], inputs_np["b3"])
    query = np.asarray(inputs_np["query"], np.float32).reshape(B, Q)
    xins = np.asarray(inputs_np["inputs"], np.float32).reshape(B, S * D)
    mask = np.asarray(inputs_np["mask"], np.int32)
    in_maps = []
    for c in range(NCORES):
        sl = slice(c * BC, (c + 1) * BC)
        m = {"query": query[sl], "inputs": xins[sl], "mask": mask[sl]}
        m.update(w)
        in_maps.append(m)

    partition_name = (
        nc.partition_id_tensor.name if nc.partition_id_tensor else None)
    in_names, out_names, out_avals, zero_outs = [], [], [], []
    for alloc in nc.m.functions[0].allocations:
        if not isinstance(alloc, _mybir.MemoryLocationSet):
            continue
        name = alloc.memorylocations[0].name
        if alloc.kind == "ExternalInput":
            if name != partition_name:
                in_names.append(name)
        elif alloc.kind == "ExternalOutput":
            out_names.append(name)
            shape = tuple(alloc.tensor_shape)
            dtype = _mybir.dt.np(alloc.dtype)
            out_avals.append(jax.core.ShapedArray(shape, dtype))
            zero_outs.append(np.zeros(shape, dtype))
    n_params = len(in_names)
    all_in_names = list(in_names) + list(out_names)
    if partition_name is not None:
        all_in_names.append(partition_name)

    def _body(*args):
        operands = list(args)
        if partition_name is not None:
            operands.append(bass2jax.partition_id_tensor())
        outs = bass2jax._bass_exec_p.bind(
            *operands,
            out_avals=tuple(out_avals),
            in_names=tuple(all_in_names),
            out_names=tuple(out_names),
            lowering_input_output_aliases=(),
            sim_require_finite=True,
            sim_require_nnan=True,
            nc=nc,
        )
        return tuple(outs)

    devices = jax.devices()[:NCORES]
    mesh = Mesh(np.asarray(devices), ("core",))
    in_specs = (PartitionSpec("core"),) * (n_params + len(out_names))
    out_specs = (PartitionSpec("core"),) * len(out_names)
    fn = jax.jit(shard_map(
        _body, mesh=mesh, in_specs=in_specs, out_specs=out_specs,
        check_rep=False))

    concat_in = [
        np.concatenate([np.asarray(in_maps[c][nm]) for c in range(NCORES)],
                       axis=0)
        for nm in in_names
    ] + [np.concatenate([z] * NCORES, axis=0) for z in zero_outs]
    sharding = jax.sharding.NamedSharding(mesh, PartitionSpec("core"))
    dev_in = [jax.device_put(a, sharding) for a in concat_in]

    outs = fn(*dev_in)  # compile + warmup
    jax.block_until_ready(outs)
    times = []
    for _ in range(iters):
        t0 = time.perf_counter()
        outs = fn(*dev_in)
        jax.block_until_ready(outs)
        times.append((time.perf_counter() - t0) * 1e9)
    return min(times), times
